# revision 13
# baseline (speedup 1.0000x reference)
"""Trainium2 Bass kernel for nn_DeepSetAttentionModel (segment_reduce).

Division of labour (device does the three dense 128x128 phi layers, ~90% of
the model FLOPs; host does O(N*small) prep and reductions):
  * Host assembles the 48-dim token features (sin/cos positional enc, value,
    one-hot measurement) and the first embedding layer h0 = relu(W0.T x + b0)
    (K=48, 11% of FLOPs) -- one [128,48]@[48,N] sgemm per core -- shipping
    h0 [128, 32768] bf16 per core to HBM.
  * The psi-MLP / segment-mean branch adds a per-segment constant per head to
    the attention logits; segment softmax is invariant to it, so the whole
    psi branch cancels and is dropped.
  * Attention logits are rank-4 per token (z = x @ M1 with M1 folded from
    W_k[:48] . W_q); host computes the segment softmax weights exactly.
  * The 64 demo tokens (1 per segment) go through the phi MLP on host in
    f32; the device stream is exactly 8 segments x 4096 time tokens per
    core -- no padding columns.
  * Device: phi layers 1-3 (128->128->128->128) in bf16, layer-serial per
    2-segment super-block so the PE streams 512-col matmuls with stationary
    weights (3 LDWEIGHTS per super-block).  PSUM->SBUF relu drains alternate
    between the Activation and Vector engines on 2-bank (1024-col) groups,
    4 groups in flight.  enc [128, 32768] bf16 streams back to HBM in
    2048-col pieces, overlapped with compute.
  * Host: attention-weighted segment sums over enc (O(N*512) f32 BLAS),
    then the tiny rho MLP 512->128->128->128->1 + sigmoid.

Sharding: data-parallel across patients -- 8 whole segments per core,
weights replicated, no collectives.
"""

import numpy as np
import ml_dtypes

import concourse.bass as bass
import concourse.tile as tile
from concourse import bacc, mybir
from concourse.bass import _add_dep_helper
from concourse.bass_utils import run_bass_kernel_spmd

F32 = mybir.dt.float32
BF16 = mybir.dt.bfloat16
FP8 = mybir.dt.float8e4
AF = mybir.ActivationFunctionType
ALU = mybir.AluOpType
NPBF16 = ml_dtypes.bfloat16

NCORES = 8
B, T = 64, 4096
SEG = 8                  # segments per core
N = SEG * T              # 32768 device tokens per core
D_IN = 48
HEADS, DOT = 4, 64
N_MOD = 37

SB_SEGS = 2              # segments per super-block
NSB = SEG // SB_SEGS     # 4 super-blocks
SBW = SB_SEGS * T        # 8192 cols per super-block
CH = 512                 # matmul moving width (one PSUM bank of f32)
GRP = 2 * CH             # drain group: 2 banks = 1024 cols
NGRP = SBW // GRP        # 8 drain groups per layer per super-block
ENC_DMA = 2 * GRP        # enc DMA piece: 2048 cols (512KB)

_CACHE = {}


def _build(zero_b):
    nc = bacc.Bacc(
        "TRN2",
        target_bir_lowering=False,
        debug=False,
        enable_asserts=False,
        num_devices=NCORES,
    )

    io = {
        "h0in": nc.dram_tensor("h0in", [128, N], BF16,
                               kind="ExternalInput").ap(),
        "wpack": nc.dram_tensor("wpack", [128, 384], BF16,
                                kind="ExternalInput").ap(),
        "cpack": nc.dram_tensor("cpack", [128, 3], F32,
                                kind="ExternalInput").ap(),
        "enc": nc.dram_tensor("enc", [128, N], FP8, kind="ExternalOutput").ap(),
    }

    with tile.TileContext(nc) as tc:
        _emit(tc, io, zero_b)

    _dedup_ldweights(nc)
    nc.compile()
    return nc


def _ldw_key(inst):
    ap = inst.ins[0]
    return (
        getattr(ap, "memref", None),
        ap.offset,
        tuple(tuple(p) for p in ap.ap),
        str(ap.dtype),
        str(getattr(inst, "tile_position", None)),
        str(getattr(inst, "perf_mode", None)),
        bool(inst.is_transpose or False),
    )


def _dedup_ldweights(nc):
    """Drop InstLdweights that reload the stationary operand already in the
    PE array (identical weights AP, no intervening PE weight writes).  The
    PE keeps weights across matmuls, so the reload is semantically a no-op
    but costs ~90ns and breaks back-to-back matmul fill/drain pipelining.
    Dropped instructions' semaphore waits transfer to the next PE
    instruction."""
    removed = 0
    for fn in nc.m.functions:
        for b in fn.blocks:
            last_key = None
            pending_waits = []
            keep = []
            for inst in b.instructions:
                eng = getattr(inst, "engine", None)
                if isinstance(inst, mybir.InstLdweights):
                    key = _ldw_key(inst)
                    si = inst.sync_info
                    if key == last_key and not (si and si.on_update):
                        if si and si.on_wait:
                            pending_waits.extend(si.on_wait)
                        removed += 1
                        continue
                    last_key = key
                elif isinstance(inst, mybir.InstMatmult):
                    if inst.ldweights:
                        last_key = None
                elif eng == mybir.EngineType.PE and not inst.is_sequencer_only():
                    last_key = None
                if pending_waits and eng == mybir.EngineType.PE:
                    si = inst.sync_info
                    if si is None:
                        inst.sync_info = mybir.SyncInfo(
                            on_wait=list(pending_waits), on_update=[])
                    else:
                        si.on_wait = list(si.on_wait) + pending_waits
                    pending_waits = []
                keep.append(inst)
            assert not pending_waits, "dropped LDW waits with no PE successor"
            b.instructions[:] = keep
    return removed


def _emit(tc, io, zero_b):
    nc = tc.nc
    sync = nc.sync
    act = nc.scalar
    dve = nc.vector
    pe = nc.tensor

    with tc.tile_pool(name="const", bufs=1) as cp:
        wsb = cp.tile([128, 384], BF16, tag="wsb")
        sync.dma_start(wsb, io["wpack"])
        csb = cp.tile([128, 3], F32, tag="csb")
        sync.dma_start(csb, io["cpack"])

        # h0 super-block tiles.  The first few pieces are small and chained
        # (priority: the first matmul starts as soon as ~256KB has landed);
        # the bulk pieces are unchained so they overlap in flight across the
        # DMA engines instead of serializing.
        hb = [cp.tile([128, SBW], BF16, tag=f"h0_{sb}", name=f"h0_{sb}")
              for sb in range(NSB)]
        pieces = [1024, 1024, 2048, 4096] + [8192] * ((NSB * SBW - 8192) // 8192)
        prev = None
        off = 0
        for pi, w_cols in enumerate(pieces):
            sb, lo = off // SBW, off % SBW
            d = sync.dma_start(hb[sb][:, lo:lo + w_cols],
                               io["h0in"][:, off:off + w_cols])
            if prev is not None and pi <= 4:
                _add_dep_helper(d.ins, prev.ins, reason="h0 dma chain")
            prev = d
            off += w_cols
        assert off == NSB * SBW

        w = [wsb[:, 0:128], wsb[:, 128:256], wsb[:, 256:384]]
        pb = [csb[:, i:i + 1] for i in range(3)]

        # Warm each drain engine during the DMA prologue: the first ACT use
        # pays a ~1.3us activation-table load, the first DVE op pays a
        # similar tensor load.
        wa = cp.tile([1, 1], F32, tag="warma")
        dve.memset(wa, 0.0)
        act.activation(wa, wa, AF.Relu)
        wv = cp.tile([1, 1], F32, tag="warmv")
        dve.memset(wv, 0.0)
        dve.tensor_scalar(wv, wv, 0.0, 0.0, ALU.add, ALU.max)

        with tc.tile_pool(name="hbuf", bufs=1) as hp, \
             tc.tile_pool(name="encb", bufs=2) as ep, \
             tc.tile_pool(name="ps", bufs=4, space="PSUM") as pp:
            for sb in range(NSB):
                h1 = hp.tile([128, SBW], BF16, tag="h1", name=f"h1_{sb}")
                h2 = hp.tile([128, SBW], BF16, tag="h2", name=f"h2_{sb}")
                enc = ep.tile([128, SBW], FP8, tag="enc", name=f"enc_{sb}")
                layers = (
                    (hb[sb], w[0], pb[0], h1),
                    (h1, w[1], pb[1], h2),
                    (h2, w[2], pb[2], enc),
                )
                for li, (src, wk, bk, dst) in enumerate(layers):
                    for g in range(NGRP):
                        ps = pp.tile([128, GRP], F32, tag="ps",
                                     name=f"ps{sb}_{li}_{g}")
                        for c in range(GRP // CH):
                            col = g * GRP + c * CH
                            pe.matmul(ps[:, c * CH:(c + 1) * CH], wk,
                                      src[:, col:col + CH],
                                      start=True, stop=True)
                        dv = dst[:, g * GRP:(g + 1) * GRP]
                        if g % 2 == 0:
                            if zero_b:
                                act.activation(dv, ps, AF.Relu)
                            else:
                                act.activation(dv, ps, AF.Relu, bias=bk)
                        elif zero_b:
                            dve.tensor_scalar_max(dv, ps, 0.0)
                        else:
                            dve.tensor_scalar(dv, ps, bk, 0.0,
                                              ALU.add, ALU.max)
                        if dst is enc and (g + 1) % (ENC_DMA // GRP) == 0:
                            base = (g + 1) * GRP - ENC_DMA
                            seg0 = sb * SB_SEGS
                            sync.dma_start(
                                io["enc"][:, seg0 * T + base:
                                          seg0 * T + base + ENC_DMA],
                                enc[:, base:base + ENC_DMA])


def get_nc(zero_b):
    key = ("nc", zero_b)
    if key not in _CACHE:
        _CACHE[key] = _build(zero_b)
    return _CACHE[key]


def _fp8_to_f32(a):
    lut = _CACHE.get("fp8lut")
    if lut is None:
        lut = np.arange(256, dtype=np.uint8).view(ml_dtypes.float8_e4m3) \
            .astype(np.float32)
        _CACHE["fp8lut"] = lut
    return lut[np.asarray(a).view(np.uint8)]


def host_prep(inputs):
    """Host-side prep: feature assembly, the K=48 embedding layer h0,
    exact attention weights, demo-token phi MLP, weight packing."""
    f32 = np.float32
    times = np.asarray(inputs["times"], f32).reshape(B, T)
    values = np.asarray(inputs["values"], f32).reshape(B, T)
    meas = np.asarray(inputs["measurements"])
    demo = np.asarray(inputs["demo"], f32)
    timescales = np.asarray(inputs["timescales"], f32)
    seg_ids = np.asarray(inputs["segment_ids"])
    expect = np.repeat(np.arange(B, dtype=seg_ids.dtype), T + 1)
    assert seg_ids.shape == expect.shape and np.array_equal(seg_ids, expect), \
        "kernel assumes full-length segments (repeat(arange(B), T+1))"

    # ---- time-token features feat [B, T, 48] ----
    scaled = times[:, :, None] / timescales[None, None, :]
    feat = np.zeros((B, T, D_IN), f32)
    feat[:, :, 0:5] = np.sin(scaled)
    feat[:, :, 5:10] = np.cos(scaled)
    feat[:, :, 10] = values
    feat[:, :, 11:48] = (meas[:, :, None] ==
                         np.arange(N_MOD)[None, None, :]).astype(f32)

    # ---- demo token: encoder + full phi MLP on host (64 tokens, f32) ----
    demo_enc = np.maximum(
        demo @ np.asarray(inputs["demo_W1"], f32)
        + np.asarray(inputs["demo_b1"], f32), 0.0) \
        @ np.asarray(inputs["demo_W2"], f32) + np.asarray(inputs["demo_b2"], f32)
    h = demo_enc
    for i in range(4):
        h = np.maximum(h @ np.asarray(inputs[f"phi_W{i}"], f32)
                       + np.asarray(inputs[f"phi_b{i}"], f32), 0.0)
    enc_demo = h                                    # [B, 128]

    # ---- attention weights: e = exp(z - max) over each 4097-token segment
    W_k = np.asarray(inputs["W_k"], f32)
    W_q = np.asarray(inputs["W_q"], f32)
    M1 = np.einsum("ihd,hd->ih", W_k[:D_IN].reshape(D_IN, HEADS, DOT),
                   W_q) / np.sqrt(f32(DOT))
    z = feat @ M1                                   # [B, T, 4]
    z_demo = demo_enc @ M1                          # [B, 4]
    m = np.maximum(z.max(axis=1), z_demo)           # [B, 4]
    e_time = np.exp(z - m[:, None, :])              # [B, T, 4]
    e_demo = np.exp(z_demo - m)                     # [B, 4]
    inv = 1.0 / (e_time.sum(axis=1) + e_demo)       # [B, 4]

    # ---- embedding layer h0 = relu(W0.T x + b0) per core, [128, N] bf16
    W0T = np.ascontiguousarray(np.asarray(inputs["phi_W0"], f32).T)
    b0 = np.asarray(inputs["phi_b0"], f32)

    wpack = np.zeros((128, 384), f32)
    wpack[:, 0:128] = np.asarray(inputs["phi_W1"], f32)
    wpack[:, 128:256] = np.asarray(inputs["phi_W2"], f32)
    wpack[:, 256:384] = np.asarray(inputs["phi_W3"], f32)
    wpack_bf = wpack.astype(NPBF16)
    cpack = np.zeros((128, 3), f32)
    for i in range(3):
        cpack[:, i] = np.asarray(inputs[f"phi_b{i + 1}"], f32)
    zero_b = bool(np.all(cpack == 0.0))

    in_maps = []
    for core in range(NCORES):
        x = feat[core * SEG:(core + 1) * SEG] \
            .transpose(2, 0, 1).reshape(D_IN, N)    # [48, N]
        h0 = np.maximum(W0T @ x + b0[:, None], 0.0)  # [128, N]
        in_maps.append({
            "h0in": h0.astype(NPBF16),
            "wpack": wpack_bf,
            "cpack": cpack,
        })
    return in_maps, e_time, e_demo, inv, enc_demo, zero_b


def finish(enc_cores, inputs, e_time, e_demo, inv, enc_demo):
    """Attention-weighted segment sums over enc + rho MLP + sigmoid."""
    f32 = np.float32
    agg = np.empty((B, HEADS, 128), f32)
    for c in range(NCORES):
        enc_f = _fp8_to_f32(enc_cores[c])                # [128, N]
        for k in range(SEG):
            s = c * SEG + k
            a = enc_f[:, k * T:(k + 1) * T] @ e_time[s]  # [128, 4]
            a += np.outer(enc_demo[s], e_demo[s])
            agg[s] = (a * inv[s][None, :]).T
    x = agg.reshape(B, HEADS * 128)
    for i in range(3):
        x = np.maximum(x @ np.asarray(inputs[f"rho_W{i}"], f32)
                       + np.asarray(inputs[f"rho_b{i}"], f32), 0.0)
    o = x @ np.asarray(inputs["rho_W3"], f32) \
        + np.asarray(inputs["rho_b3"], f32)
    return (1.0 / (1.0 + np.exp(-o.astype(np.float64)))).astype(f32)


def kernel(**inputs):
    in_maps, e_time, e_demo, inv, enc_demo, zero_b = host_prep(inputs)
    nc = get_nc(zero_b)
    res = run_bass_kernel_spmd(nc, in_maps, core_ids=list(range(NCORES)))
    enc_cores = [res.results[c]["enc"] for c in range(NCORES)]
    return finish(enc_cores, inputs, e_time, e_demo, inv, enc_demo)
